# revision 4
# baseline (speedup 1.0000x reference)
"""Trainium2 Bass kernel for nn_LL1_63178968924376 (dense_cnn).

Single-channel CNN pyramid on (16,1,1024,1024) -> (16,1,1,1).
Data parallel: 2 images per core across 8 NeuronCores.

Convs are computed on the tensor engine as banded (Toeplitz) matmuls over the
H (partition) axis, one matmul per kernel column, with the W-shift taken as a
free-axis offset into a zero-padded SBUF tile.  float32r operands give
1 cycle/row.  The 1x1-conv + BatchNorm + tanh tail of every branch folds into
a single ScalarE activation (tanh(alpha*psum + beta)) on PSUM eviction; the
elementwise branch multiply runs on VectorE; 4x4 avg-pool = VectorE strided
W-reduce + a small pooling matmul over partitions.
"""
import contextlib
import numpy as np

import concourse.bacc as bacc
import concourse.mybir as mybir
import concourse.tile as tile
from concourse.bass_utils import run_bass_kernel_spmd

F32 = mybir.dt.float32
F32R = mybir.dt.float32r
EPS = 1e-5
NCORES = 8
IMGS_PER_CORE = 2
NRHS = 3


def _band(w2d, dil):
    """[K,128,128] Toeplitz bands: band[j][m + i*dil, m] = w[i,j]."""
    K = w2d.shape[0]
    b = np.zeros((K, 128, 128), np.float32)
    for j in range(K):
        for i in range(K):
            q0 = i * dil
            for m in range(128 - q0):
                b[j, m + q0, m] = w2d[i, j]
    return b


class _Pass:
    def __init__(self, name, H, K, dil, pad, kind, in_buf, out_buf,
                 scale=1.0, bias=0.0, mul_buf=None, pool=False):
        self.name, self.H, self.K, self.dil, self.pad = name, H, K, dil, pad
        self.kind = kind  # 'pre' | 'tanh'
        self.in_buf, self.out_buf, self.mul_buf = in_buf, out_buf, mul_buf
        self.scale, self.bias, self.pool = scale, bias, pool
        keff = (K - 1) * dil + 1
        m = min(128 - (keff - 1), H)
        if pool:
            m -= m % 4
        self.M_out = m
        self.band_off = None  # filled at pack time


def _make_passes(params):
    p = {k: {kk: np.asarray(vv, np.float32) for kk, vv in v.items()}
         for k, v in params.items()}

    def br_consts(c):
        w3 = float(c['w3'].reshape(()))
        b3 = float(c['b3'].reshape(()))
        b2 = float(c['b2'].reshape(()))
        g = float(c['g'].reshape(()))
        be = float(c['be'].reshape(()))
        m = float(c['m'].reshape(()))
        v = float(c['v'].reshape(()))
        rs = 1.0 / np.sqrt(v + EPS)
        alpha = w3 * g * rs
        beta = (w3 * b2 + b3 - m) * g * rs + be
        return alpha, beta

    def c4_consts(c, i):
        b = float(c[f'b{i}'].reshape(()))
        g = float(c[f'g{i}'].reshape(()))
        be = float(c[f'be{i}'].reshape(()))
        m = float(c[f'm{i}'].reshape(()))
        v = float(c[f'v{i}'].reshape(()))
        rs = 1.0 / np.sqrt(v + EPS)
        return g * rs, (b - m) * g * rs + be

    passes, bands = [], []

    def add(name, H, K, dil, pad, kind, in_buf, out_buf, w2d,
            scale=1.0, bias=0.0, mul_buf=None, pool=False):
        ps = _Pass(name, H, K, dil, pad, kind, in_buf, out_buf,
                   scale, bias, mul_buf, pool)
        ps.band_off = sum(b.shape[0] for b in bands)
        bands.append(_band(w2d, dil))
        passes.append(ps)
        return ps

    # stage 1 (1024)
    a1, be1 = br_consts(p['c1'])
    add("s1a", 1024, 7, 1, 3, 'pre', 'x', 't1024',
        p['c1']['w1'][0, 0], bias=float(p['c1']['b1'].reshape(())))
    add("s1b", 1024, 9, 4, 16, 'tanh', 't1024', 'x1',
        p['c1']['w2'][0, 0], scale=a1, bias=be1, mul_buf='x')
    a2, be2 = br_consts(p['c11'])
    add("s1c", 1024, 7, 1, 3, 'pre', 'x1', 't1024',
        p['c11']['w1'][0, 0], bias=float(p['c11']['b1'].reshape(())))
    add("s1d", 1024, 9, 4, 16, 'tanh', 't1024', 'x2',
        p['c11']['w2'][0, 0], scale=a2, bias=be2, mul_buf='x1', pool=True)
    # stage 2 (256)
    a3, be3 = br_consts(p['c2'])
    add("s2a", 256, 5, 1, 2, 'pre', 'x2', 't256',
        p['c2']['w1'][0, 0], bias=float(p['c2']['b1'].reshape(())))
    add("s2b", 256, 7, 3, 9, 'tanh', 't256', 'x2b',
        p['c2']['w2'][0, 0], scale=a3, bias=be3, mul_buf='x2')
    a4, be4 = br_consts(p['c22'])
    add("s2c", 256, 5, 1, 2, 'pre', 'x2b', 't256',
        p['c22']['w1'][0, 0], bias=float(p['c22']['b1'].reshape(())))
    add("s2d", 256, 7, 3, 9, 'tanh', 't256', 'x3',
        p['c22']['w2'][0, 0], scale=a4, bias=be4, mul_buf='x2b', pool=True)
    # stage 3 (64); NOTE c33 branch reads x3 (not the c3 product)
    a5, be5 = br_consts(p['c3'])
    add("s3a", 64, 5, 1, 2, 'pre', 'x3', 't64',
        p['c3']['w1'][0, 0], bias=float(p['c3']['b1'].reshape(())))
    add("s3b", 64, 5, 3, 6, 'tanh', 't64', 'w3t',
        p['c3']['w2'][0, 0], scale=a5, bias=be5, mul_buf='x3')
    a6, be6 = br_consts(p['c33'])
    add("s3c", 64, 5, 1, 2, 'pre', 'x3', 't64',
        p['c33']['w1'][0, 0], bias=float(p['c33']['b1'].reshape(())))
    add("s3d", 64, 5, 3, 6, 'tanh', 't64', 'y0',
        p['c33']['w2'][0, 0], scale=a6, bias=be6, mul_buf='w3t', pool=True)
    # stage 4 (16): tanh(bn(conv)) twice
    a7, be7 = c4_consts(p['c4'], 1)
    add("s4a", 16, 3, 1, 1, 'tanh', 'y0', 'g1',
        p['c4']['w1'][0, 0], scale=a7, bias=be7)
    a8, be8 = c4_consts(p['c4'], 2)
    add("s4b", 16, 3, 1, 1, 'tanh', 'g1', 'g2',
        p['c4']['w2'][0, 0], scale=a8, bias=be8)

    return passes, np.concatenate(bands, axis=0)


def _build_program(passes, n_bands):
    nc = bacc.Bacc("TRN2", target_bir_lowering=False, debug=False)
    x_d = nc.dram_tensor("xin", [IMGS_PER_CORE, 1024, 1024], F32R,
                         kind="ExternalInput")
    b_d = nc.dram_tensor("bands", [n_bands, 128, 128], F32R,
                         kind="ExternalInput")
    z_d = nc.dram_tensor("zeros", [128, 1056], F32R, kind="ExternalInput")
    pb_d = nc.dram_tensor("poolb", [128, 32], F32, kind="ExternalInput")
    o_d = nc.dram_tensor("out", [IMGS_PER_CORE, 1], F32, kind="ExternalOutput")

    with tile.TileContext(nc) as tc:
        with contextlib.ExitStack() as ctx:
            singles = ctx.enter_context(tc.tile_pool(name="singles", bufs=1))
            psum_pool = ctx.enter_context(
                tc.tile_pool(name="psum", bufs=2, space="PSUM"))
            out_pool = ctx.enter_context(tc.tile_pool(name="outp", bufs=3))
            aux_pool = ctx.enter_context(tc.tile_pool(name="aux", bufs=3))
            dram = ctx.enter_context(
                tc.tile_pool(name="dram", bufs=1, space="DRAM"))

            # all Toeplitz bands in one SBUF tile, one DMA
            bands_sb = singles.tile([128, n_bands * 128], F32R)
            nc.sync.dma_start(
                out=bands_sb.rearrange("p (n m) -> p n m", n=n_bands),
                in_=b_d[:].rearrange("n p m -> p n m"))
            # 4x4 avg-pool matrix (1/16 entries), host-provided
            poolb = singles.tile([128, 32], F32)
            nc.sync.dma_start(out=poolb[:, :], in_=pb_d[:, :])

            # per-pass activation bias values as [128,1] columns of one tile
            biases = singles.tile([128, len(passes) + 1], F32)
            for i, ps in enumerate(passes):
                nc.vector.memset(biases[:, i:i + 1], ps.bias)
            ZB = len(passes)
            nc.vector.memset(biases[:, ZB:ZB + 1], 0.0)

            # pre-zeroed rhs tile sets keyed by (H, pad)
            rhs_sets = {}
            for ps in passes:
                key = (ps.H, ps.pad)
                if key in rhs_sets:
                    continue
                W = ps.H
                WP = W + 2 * ps.pad
                n_blocks = (ps.H + ps.M_out - 1) // ps.M_out
                n_int = min(NRHS, max(n_blocks - 2, 0))
                tiles = []
                for i in range(n_int + 2):
                    t = singles.tile([128, WP], F32R, tag=f"rhs{key}_{i}",
                                     name=f"rhs{key}_{i}")
                    nc.sync.dma_start(out=t[:, :], in_=z_d[:, :WP])
                    tiles.append(t)
                rhs_sets[key] = tiles

            def pick_rhs(key, c, n_blocks):
                tiles = rhs_sets[key]
                if c == 0:
                    return tiles[-2]
                if c == n_blocks - 1:
                    return tiles[-1]
                return tiles[(c - 1) % (len(tiles) - 2)]

            bufs = {}

            def dram_buf(name, img, H, W):
                k = (name, img)
                if k not in bufs:
                    bufs[k] = dram.tile([H, W], F32R, tag=f"{name}{img}",
                                        name=f"{name}{img}")
                return bufs[k]

            def conv_pass(ps, img):
                H = W = ps.H
                K, dil, pad, M_out = ps.K, ps.dil, ps.pad, ps.M_out
                bidx = passes.index(ps)
                if ps.in_buf == 'x':
                    in_d = x_d[img]
                else:
                    in_d = dram_buf(ps.in_buf, img, H, W)
                out_H = H // 4 if ps.pool else H
                last_sb = None
                if ps.name != "s4b":
                    out_d = dram_buf(ps.out_buf, img, out_H, out_H)
                n_blocks = (H + M_out - 1) // M_out
                for c in range(n_blocks):
                    r0 = c * M_out
                    M = min(M_out, H - r0)
                    rhs = pick_rhs((H, pad), c, n_blocks)
                    s = r0 - pad
                    r_lo, r_hi = max(0, s), min(H, s + 128)
                    q_lo, q_hi = r_lo - s, r_hi - s
                    nc.sync.dma_start(out=rhs[q_lo:q_hi, pad:pad + W],
                                      in_=in_d[r_lo:r_hi, :])
                    psum = psum_pool.tile([128, W], F32, tag="psum",
                                          name="psum")
                    for h in range((W + 511) // 512):
                        n0 = h * 512
                        N = min(512, W - n0)
                        for j in range(K):
                            off = (ps.band_off + j) * 128
                            nc.tensor.matmul(
                                psum[:M, n0:n0 + N],
                                bands_sb[:, off:off + M],
                                rhs[:, n0 + j * dil:n0 + j * dil + N],
                                start=(j == 0), stop=(j == K - 1))
                    func = (mybir.ActivationFunctionType.Tanh
                            if ps.kind == 'tanh'
                            else mybir.ActivationFunctionType.Identity)
                    mul = ps.mul_buf is not None
                    act_sb = out_pool.tile([128, W], F32 if mul else F32R,
                                           tag="act_sb", name="act_sb")
                    nc.scalar.activation(out=act_sb[:M, :], in_=psum[:M, :],
                                         func=func, bias=biases[:M, bidx:bidx + 1],
                                         scale=float(ps.scale))
                    res_sb = act_sb
                    if mul:
                        md = (x_d[img] if ps.mul_buf == 'x'
                              else dram_buf(ps.mul_buf, img, H, W))
                        xm = aux_pool.tile([128, W], F32, tag="xm", name="xm")
                        nc.sync.dma_start(out=xm[:M, :],
                                          in_=md[r0:r0 + M, :].bitcast(F32))
                        res_sb = out_pool.tile([128, W], F32R, tag="mul_sb",
                                               name="mul_sb")
                        nc.vector.tensor_mul(res_sb[:M, :], act_sb[:M, :],
                                             xm[:M, :])
                    if not ps.pool:
                        if ps.name == "s4b":
                            last_sb = res_sb
                        else:
                            nc.sync.dma_start(out=out_d[r0:r0 + M, :],
                                              in_=res_sb[:M, :])
                    else:
                        wp = aux_pool.tile([128, W // 4], F32, tag="wp",
                                           name="wp")
                        nc.vector.reduce_sum(
                            out=wp[:M, :],
                            in_=res_sb[:M, :].bitcast(F32).rearrange(
                                "p (a b) -> p a b", b=4),
                            axis=mybir.AxisListType.X)
                        MP = M // 4
                        psum2 = psum_pool.tile([32, W // 4], F32, tag="psum2",
                                               name="psum2")
                        nc.tensor.matmul(psum2[:MP, :], poolb[:M, :MP],
                                         wp[:M, :], start=True, stop=True)
                        pool_sb = aux_pool.tile([32, W // 4], F32R,
                                                tag="pool_sb", name="pool_sb")
                        nc.scalar.activation(
                            out=pool_sb[:MP, :], in_=psum2[:MP, :],
                            func=mybir.ActivationFunctionType.Identity,
                            bias=biases[:MP, ZB:ZB + 1], scale=1.0)
                        nc.sync.dma_start(out=out_d[r0 // 4:r0 // 4 + MP, :],
                                          in_=pool_sb[:MP, :])
                return last_sb

            # emit: passes outer, images inner (overlaps the two images'
            # dependency chains across pass boundaries)
            g2 = [None] * IMGS_PER_CORE
            for ps in passes:
                for img in range(IMGS_PER_CORE):
                    r = conv_pass(ps, img)
                    if ps.name == "s4b":
                        g2[img] = r

            # finale: global mean over 16x16 + sigmoid
            for img in range(IMGS_PER_CORE):
                red = aux_pool.tile([16, 1], F32, tag=f"red{img}",
                                    name=f"red{img}")
                nc.vector.reduce_sum(out=red[:16, :],
                                     in_=g2[img][:16, :16].bitcast(F32),
                                     axis=mybir.AxisListType.X)
                sc = dram.tile([16], F32, tag=f"sc{img}", name=f"sc{img}")
                nc.sync.dma_start(out=sc[:], in_=red[:16, :])
                row = aux_pool.tile([1, 16], F32, tag=f"row{img}",
                                    name=f"row{img}")
                nc.sync.dma_start(out=row[:1, :], in_=sc[:])
                tot = aux_pool.tile([1, 1], F32, tag=f"tot{img}",
                                    name=f"tot{img}")
                nc.vector.reduce_sum(out=tot[:1, :], in_=row[:1, :],
                                     axis=mybir.AxisListType.X)
                res = aux_pool.tile([1, 1], F32, tag=f"res{img}",
                                    name=f"res{img}")
                nc.scalar.activation(out=res[:1, :], in_=tot[:1, :],
                                     func=mybir.ActivationFunctionType.Sigmoid,
                                     bias=biases[:1, ZB:ZB + 1],
                                     scale=1.0 / 256.0)
                nc.sync.dma_start(out=o_d[img:img + 1, :], in_=res[:1, :])

    nc.compile()
    return nc


_CACHE = {}
TRACE = False
LAST_EXEC_NS = None


def kernel(x, params):
    x = np.ascontiguousarray(np.asarray(x, np.float32))
    passes, bands = _make_passes(params)
    key = "prog"
    if key not in _CACHE:
        _CACHE[key] = _build_program(passes, bands.shape[0])
    nc = _CACHE[key]
    zeros = np.zeros((128, 1056), np.float32)
    poolb = np.zeros((128, 32), np.float32)
    for m in range(32):
        poolb[4 * m:4 * m + 4, m] = 1.0 / 16.0
    imgs = x[:, 0]  # (16, 1024, 1024)
    in_maps = []
    for c in range(NCORES):
        in_maps.append({
            "xin": np.ascontiguousarray(imgs[c * IMGS_PER_CORE:(c + 1) * IMGS_PER_CORE]),
            "bands": bands,
            "zeros": zeros,
            "poolb": poolb,
        })
    global LAST_EXEC_NS
    res = run_bass_kernel_spmd(nc, in_maps, core_ids=list(range(NCORES)),
                               trace=TRACE)
    LAST_EXEC_NS = res.exec_time_ns
    outs = [r["out"] for r in res.results]  # each [2,1]
    return np.concatenate(outs, axis=0).reshape(16, 1, 1, 1).astype(np.float32)
